# revision 16
# baseline (speedup 1.0000x reference)
"""TRN2 Bass kernel for nn_BeliefUpdater (scatter_memory).

Sharding: pure data-parallel over batch (256 -> 8 cores x 32), weights replicated.

Precision strategy (driven by top-k rank-flip sensitivity; min adjacent-score
gap in the reference data is ~2e-6):
  - score path (ad/rd MLPs -> new_*_scores -> top_k): fp32-grade via 3-term
    f32r hi/lo split at 11 explicit mantissa bits (f32r is exact at 11 bits
    and runs at 1 cycle/row vs fp32's 4).
  - slot path (au/ru MLPs): single-pass f32r (~1e-4 rel err) + bf16 second
    layer; residual add in fp32.
  - gelu: ScalarE LUT (erf-grade, ~2e-6 max err).

Layout: first layers run feature-on-partition ("T layout") so the per-batch
context bias folds into the gelu eviction (ACT per-partition bias, one 64-token
batch group at a time); second layers flip back to token-major via
lhsT=gelu-output. Tokens are processed in 2 halves of 1024 to fit SBUF.
"""
import sys
import numpy as np

sys.path.insert(0, "/opt/trn_rl_repo")

NCORES = 8
B = 32          # batches per core
D = 1024
KSLOT = 64      # approach/rule slots per batch
A = 16          # archive slots
T = B * KSLOT   # tokens per side per core = 2048
TH = T // 2     # tokens per half = 1024
TT = T // 128   # token tiles total = 16
TTH = TH // 128  # token tiles per half = 8
H_AD = 2048
H_AU = 1024
NB = 11         # f32r-exact mantissa bits


def _trunc(x, nb=NB):
    xi = np.ascontiguousarray(x, dtype=np.float32).view(np.uint32)
    mask = np.uint32(0xFFFFFFFF) << np.uint32(23 - nb)
    return (xi & mask).view(np.float32)


def _split(x):
    h = _trunc(x)
    return h, np.ascontiguousarray((x - h).astype(np.float32))


def build_nc(scalars):
    import concourse.bass as bass
    import concourse.tile as tile
    from concourse import mybir, bacc
    from concourse.masks import make_identity
    from contextlib import ExitStack

    F32 = mybir.dt.float32
    R32 = mybir.dt.float32r
    BF16 = mybir.dt.bfloat16
    AF = mybir.ActivationFunctionType
    ALU = mybir.AluOpType
    AX = mybir.AxisListType

    nc = bacc.Bacc("TRN2", target_bir_lowering=False, debug=False, num_devices=NCORES)

    def din(name, shape, dt=F32):
        return nc.dram_tensor(name, shape, dt, kind="ExternalInput")

    def dout(name, shape, dt=F32):
        return nc.dram_tensor(name, shape, dt, kind="ExternalOutput")

    xa = din("xa", [T, D]); xr = din("xr", [T, D])
    xta_h = din("xta_h", [D, T]); xta_l = din("xta_l", [D, T])
    xtr_h = din("xtr_h", [D, T]); xtr_l = din("xtr_l", [D, T])
    sca_tok = din("sca_tok", [128, TT]); scr_tok = din("scr_tok", [128, TT])
    arch_slots = din("arch_slots", [B, A, D]); arch_sc = din("arch_sc", [B, A])
    ctxT = din("ctxT", [D, B]); cpT = din("cpT", [D, B]); obT = din("obT", [D, B])
    Wp = din("Wp", [D, D]); Wo = din("Wo", [D, D])
    bp_t = din("bp_t", [128, 8]); bo_t = din("bo_t", [128, 8])
    Wctx_au = din("Wctx_au", [3 * D, H_AU]); Wctx_ru = din("Wctx_ru", [3 * D, H_AU])
    Wctx_ad = din("Wctx_ad", [3 * D, H_AD]); Wctx_rd = din("Wctx_rd", [3 * D, H_AD])
    b1au_t = din("b1au_t", [128, 8]); b1ru_t = din("b1ru_t", [128, 8])
    b1ad_t = din("b1ad_t", [128, 16]); b1rd_t = din("b1rd_t", [128, 16])
    W1s_au = din("W1s_au", [D, H_AU]); W1s_ru = din("W1s_ru", [D, H_AU])
    W1s_ad_h = din("W1s_ad_h", [D, H_AD]); W1s_ad_l = din("W1s_ad_l", [D, H_AD])
    W1s_rd_h = din("W1s_rd_h", [D, H_AD]); W1s_rd_l = din("W1s_rd_l", [D, H_AD])
    W2au = din("W2au", [H_AU, D], BF16); W2ru = din("W2ru", [H_AU, D], BF16)
    b2au_row = din("b2au_row", [1, D], BF16); b2ru_row = din("b2ru_row", [1, D], BF16)
    W2ad_t = din("W2ad_t", [128, 16]); W2rd_t = din("W2rd_t", [128, 16])
    s_W1 = din("s_W1", [3 * D, D]); st_W1 = din("st_W1", [3 * D, D])
    s_b1_t = din("s_b1_t", [128, 8]); st_b1_t = din("st_b1_t", [128, 8])
    s_W2_t = din("s_W2_t", [128, 8]); st_W2_t = din("st_W2_t", [128, 8])

    o_a_slots = dout("o_a_slots", [T, D]); o_r_slots = dout("o_r_slots", [T, D])
    o_a_sc = dout("o_a_sc", [TT, 128]); o_r_sc = dout("o_r_sc", [TT, 128])
    o_top_slots = dout("o_top_slots", [B, A, D])
    o_top_vals = dout("o_top_vals", [B, A])
    o_surprise = dout("o_surprise", [1, B]); o_stag = dout("o_stag", [1, B])
    idx_scratch = nc.dram_tensor("idx_scratch", [1, B * A], F32)
    pw_scr_a = nc.dram_tensor("pw_scr_a", [TT * 128, B], F32)
    t_scr = nc.dram_tensor("t_scr", [2 * D, B], F32)
    pooled_scr = nc.dram_tensor("pooled_scr", [2 * D, B], F32)
    pw_scr_r = nc.dram_tensor("pw_scr_r", [TT * 128, B], F32)
    s_scr_a = nc.dram_tensor("s_scr_a", [A, B * A], F32)
    s_scr_s = nc.dram_tensor("s_scr_s", [KSLOT, B * A], F32)

    def r32(ap):
        return ap.bitcast(R32)

    with tile.TileContext(nc) as tc, ExitStack() as ctx:
        const = ctx.enter_context(tc.tile_pool(name="const", bufs=1))
        small = ctx.enter_context(tc.tile_pool(name="small", bufs=1))
        xtp = ctx.enter_context(tc.tile_pool(name="xtp", bufs=1))
        wstream = ctx.enter_context(tc.tile_pool(name="wstream", bufs=2))
        g2p = ctx.enter_context(tc.tile_pool(name="g2p", bufs=1))
        g1p = ctx.enter_context(tc.tile_pool(name="g1p", bufs=8))
        strm = ctx.enter_context(tc.tile_pool(name="strm", bufs=2))
        strm1 = ctx.enter_context(tc.tile_pool(name="strm1", bufs=1))
        bmrhs = ctx.enter_context(tc.tile_pool(name="bmrhs", bufs=6))
        psum = ctx.enter_context(tc.tile_pool(name="psum", bufs=2, space="PSUM"))
        psum1 = ctx.enter_context(tc.tile_pool(name="psum1", bufs=1, space="PSUM"))

        ident = const.tile([128, 128], F32, tag="ident")
        make_identity(nc, ident)
        ones_row = const.tile([1, 128], BF16, tag="ones_row")
        nc.vector.memset(ones_row, 1.0)

        # PSUM tags:
        #  psum  "mm512" [128,512] bufs2 -> 2 banks   (L1/L2 matmul chains)
        #  psum  "ps_small" [128,128] bufs2 -> 2 banks (ctx matmuls, transposes)
        #  psum  "gps" [16,512] ... moved to psum1 bufs1 -> 1 bank
        #  psum1 "acc_vec" [128,16-ish] -> 1 bank      (scores / surprise / stag)
        #  psum1 "pooled_ps" [32,1024] -> 2 banks
        def t_orient_mm(lhsT_dram, rhs_tiles, m_tiles, kt_n, dt_mm, bias_sb, func, out_tag, out_dt=F32):
            outs = []
            lhsT_r = lhsT_dram.rearrange("(a b) h -> b a h", b=128)
            for m in range(m_tiles):
                acc = psum.tile([128, B], F32, tag="ps_small")
                for kc in range(kt_n // 8):
                    lt = wstream.tile([128, 8, 128], F32, tag="ctx_w3")
                    nc.sync.dma_start(
                        out=lt, in_=lhsT_r[:, kc * 8:(kc + 1) * 8, m * 128:(m + 1) * 128])
                    for kl in range(8):
                        kt = kc * 8 + kl
                        nc.tensor.matmul(out=acc, lhsT=lt[:, kl], rhs=rhs_tiles[kt][:],
                                         start=(kt == 0), stop=(kt == kt_n - 1))
                ot = small.tile([128, B], out_dt, tag=f"{out_tag}{m}")
                nc.scalar.activation(out=ot, in_=acc, func=func,
                                     bias=bias_sb[:, m:m + 1] if bias_sb is not None else 0.0)
                outs.append(ot)
            return outs

        def load_T(dram):
            t_ = small.tile([128, 8, B], F32, tag=f"ld_{dram.name}")
            nc.sync.dma_start(out=t_, in_=dram.rearrange("(a b) n -> b a n", b=128))
            return [t_[:, kt] for kt in range(8)]

        def load_small(dram, shape, tag, dt=F32):
            t_ = small.tile(shape, dt, tag=tag)
            nc.sync.dma_start(out=t_, in_=dram[:, :])
            return t_

        ctxT_t = load_T(ctxT); cpT_t = load_T(cpT); obT_t = load_T(obT)
        bp_sb = load_small(bp_t, [128, 8], "bp_sb")
        bo_sb = load_small(bo_t, [128, 8], "bo_sb")

        probeT = t_orient_mm(Wp, cpT_t, 8, 8, F32, bp_sb, AF.Identity, "probeT")
        obsT = t_orient_mm(Wo, obT_t, 8, 8, F32, bo_sb, AF.Identity, "obsT")
        ctx3 = probeT + obsT + ctxT_t  # a_in ctx order: [probe, obs, ctx]

        # round-trip probe/obs T tiles through DRAM so they can be consumed
        # as f32r (BIR requires f32r-written producers); ctx comes from DRAM input.
        for m in range(8):
            nc.sync.dma_start(out=t_scr[m * 128:(m + 1) * 128, :], in_=probeT[m][:])
            nc.sync.dma_start(out=t_scr[D + m * 128:D + (m + 1) * 128, :], in_=obsT[m][:])
        pT_r, oT_r, cT_r = [], [], []
        for m in range(8):
            t1 = small.tile([128, B], R32, tag=f"pTr{m}")
            nc.sync.dma_start(out=t1, in_=t_scr[m * 128:(m + 1) * 128, :].bitcast(R32))
            pT_r.append(t1)
            t2 = small.tile([128, B], R32, tag=f"oTr{m}")
            nc.sync.dma_start(out=t2, in_=t_scr[D + m * 128:D + (m + 1) * 128, :].bitcast(R32))
            oT_r.append(t2)
            t3 = small.tile([128, B], R32, tag=f"cTr{m}")
            nc.sync.dma_start(out=t3, in_=ctxT[m * 128:(m + 1) * 128, :].bitcast(R32))
            cT_r.append(t3)
        ctx3_r = pT_r + oT_r + cT_r

        def bm_orient_mm(Wrhs_dram, hidden, lhsT_tiles, b1_sb, func, out_tag):
            """Batch-major f32r bias matmul (N=512) + transpose back to T tiles."""
            outs = []
            for nch in range(hidden // 512):
                acc = psum.tile([B, 512], F32, tag="pool_part")
                for kt in range(len(lhsT_tiles)):
                    rt = bmrhs.tile([128, 512], R32, tag="bm_rhs")
                    nc.sync.dma_start(
                        out=rt,
                        in_=Wrhs_dram[kt * 128:(kt + 1) * 128, nch * 512:(nch + 1) * 512].bitcast(R32))
                    nc.tensor.matmul(out=acc, lhsT=lhsT_tiles[kt][:], rhs=rt,
                                     start=(kt == 0), stop=(kt == len(lhsT_tiles) - 1))
                bm = small.tile([B, 512], F32, tag="bm_stage")
                nc.scalar.activation(out=bm, in_=acc, func=AF.Copy)
                for j in range(4):
                    m = nch * 4 + j
                    tp = psum.tile([128, 128], F32, tag="ps_small")
                    nc.tensor.transpose(out=tp[0:128, 0:B], in_=bm[:, j * 128:(j + 1) * 128],
                                        identity=ident[0:B, 0:B])
                    ot = small.tile([128, B], F32, tag=f"{out_tag}{m}")
                    nc.scalar.activation(out=ot, in_=tp[0:128, 0:B], func=func,
                                         bias=b1_sb[:, m:m + 1])
                    outs.append(ot)
            return outs

        bias_au = bm_orient_mm(Wctx_au, H_AU, ctx3_r,
                               load_small(b1au_t, [128, 8], "b1au_sb"), AF.Identity, "bias_au")
        bias_ad = t_orient_mm(Wctx_ad, ctx3, 16, 24, F32,
                              load_small(b1ad_t, [128, 16], "b1ad_sb"), AF.Identity, "bias_ad")
        bias_ru = bm_orient_mm(Wctx_ru, H_AU, ctx3_r,
                               load_small(b1ru_t, [128, 8], "b1ru_sb"), AF.Identity, "bias_ru")
        bias_rd = t_orient_mm(Wctx_rd, ctx3, 16, 24, F32,
                              load_small(b1rd_t, [128, 16], "b1rd_sb"), AF.Identity, "bias_rd")

        # ---------- surprise ----------
        s_in_r = cT_r + pT_r + oT_r   # [ctx, probe, obs]
        s_h1 = bm_orient_mm(s_W1, D, s_in_r,
                            load_small(s_b1_t, [128, 8], "s_b1_sb"), AF.Gelu, "s_h1")
        s_W2_sb = load_small(s_W2_t, [128, 8], "s_W2_sb")
        sur_acc = psum1.tile([128, B], F32, tag="acc_vec")
        for m in range(8):
            nc.tensor.matmul(out=sur_acc[0:1, :], lhsT=s_W2_sb[:, m:m + 1], rhs=s_h1[m][:],
                             start=(m == 0), stop=(m == 7))
        sur_sb = small.tile([1, B], F32, tag="sur_sb")
        nc.scalar.activation(out=sur_sb, in_=sur_acc[0:1, :], func=AF.Identity,
                             bias=float(scalars["s_b2"]))
        nc.sync.dma_start(out=o_surprise[:, :], in_=sur_sb)

        # ---------- main per-side ----------
        def do_side(xt_h_d, xt_l_d, x_d, sc_tok_d, W1s_hi_d, W1s_lo_d, W1s_au_d,
                    W2ad_td, W2au_d, b2row_d, bias_hi, bias_u, b2_scalar,
                    o_slots_d, o_sc_d, side):
            W2ad_sb = load_small(W2ad_td, [128, 16], "W2ad_sb")
            b2r_sb = small.tile([1, D], BF16, tag="b2r_sb")
            nc.sync.dma_start(out=b2r_sb, in_=b2row_d[:, :])
            xth_r = xt_h_d.rearrange("(a b) t -> b a t", b=128)
            xtl_r = xt_l_d.rearrange("(a b) t -> b a t", b=128)
            whi_r = W1s_hi_d.rearrange("(a b) h -> b a h", b=128)
            wlo_r = W1s_lo_d.rearrange("(a b) h -> b a h", b=128)
            wau_r = W1s_au_d.rearrange("(a b) h -> b a h", b=128)

            sc_sb = small.tile([128, TT], F32, tag="sc_sb")
            nc.vector.memset(sc_sb, 0.0)
            pooled_sb = small.tile([B, D], F32, tag="pooled_sb")
            nc.vector.memset(pooled_sb, 0.0)
            for half in range(2):
                xt_h = xtp.tile([128, 8, TH], R32, tag="xt_h")
                xt_l = xtp.tile([128, 8, TH], R32, tag="xt_l")
                nc.sync.dma_start(out=xt_h, in_=xth_r[:, :, half * TH:(half + 1) * TH].bitcast(R32))
                nc.sync.dma_start(out=xt_l, in_=xtl_r[:, :, half * TH:(half + 1) * TH].bitcast(R32))

                # --- AD (score) phase: hidden m-loop
                for m in range(16):
                    wh = wstream.tile([128, 8, 128], R32, tag="ad_wh")
                    wl = wstream.tile([128, 8, 128], R32, tag="ad_wl")
                    nc.sync.dma_start(out=wh, in_=whi_r[:, :, m * 128:(m + 1) * 128].bitcast(R32))
                    nc.sync.dma_start(out=wl, in_=wlo_r[:, :, m * 128:(m + 1) * 128].bitcast(R32))
                    g2 = g2p.tile([128, TH], F32, tag="g2")
                    for c in range(2):
                        h1 = psum.tile([128, 512], F32, tag="mm512")
                        n_mm = 0
                        for lt, rt in ((wh, xt_h), (wh, xt_l), (wl, xt_h)):
                            for kt in range(8):
                                nc.tensor.matmul(
                                    out=h1, lhsT=lt[:, kt],
                                    rhs=rt[:, kt, c * 512:(c + 1) * 512],
                                    start=(n_mm == 0), stop=(n_mm == 23))
                                n_mm += 1
                        b0 = half * 16 + c * 8
                        nc.vector.tensor_tensor(
                            out=h1.rearrange("p (g k) -> p g k", g=8),
                            in0=h1.rearrange("p (g k) -> p g k", g=8),
                            in1=bias_hi[m][:, b0:b0 + 8].broadcast_to([128, 8, 64]),
                            op=ALU.add)
                        nc.scalar.activation(out=g2[:, c * 512:(c + 1) * 512], in_=h1,
                                             func=AF.Gelu)
                    scp = psum.tile([128, TTH], F32, tag="ps_small")
                    for tt in range(TTH):
                        nc.tensor.matmul(out=scp[:, tt:tt + 1],
                                         lhsT=g2[:, tt * 128:(tt + 1) * 128],
                                         rhs=W2ad_sb[:, m:m + 1],
                                         start=True, stop=True)
                    nc.vector.tensor_tensor(out=sc_sb[:, half * TTH:(half + 1) * TTH],
                                            in0=sc_sb[:, half * TTH:(half + 1) * TTH],
                                            in1=scp, op=ALU.add)

                # --- AU (slot) phase
                g1 = []
                for m in range(8):
                    wau = wstream.tile([128, 8, 128], R32, tag="au_w")
                    nc.sync.dma_start(out=wau, in_=wau_r[:, :, m * 128:(m + 1) * 128].bitcast(R32))
                    g1m = g1p.tile([128, TH], BF16, tag="g1")
                    for c in range(2):
                        h1 = psum.tile([128, 512], F32, tag="mm512")
                        for kt in range(8):
                            nc.tensor.matmul(out=h1, lhsT=wau[:, kt],
                                             rhs=xt_h[:, kt, c * 512:(c + 1) * 512],
                                             start=(kt == 0), stop=(kt == 7))
                        b0 = half * 16 + c * 8
                        nc.vector.tensor_tensor(
                            out=h1.rearrange("p (g k) -> p g k", g=8),
                            in0=h1.rearrange("p (g k) -> p g k", g=8),
                            in1=bias_u[m][:, b0:b0 + 8].broadcast_to([128, 8, 64]),
                            op=ALU.add)
                        nc.scalar.activation(out=g1m[:, c * 512:(c + 1) * 512], in_=h1,
                                             func=AF.Gelu)
                    g1.append(g1m)
                for c2 in range(2):
                    w2au_sb = small.tile([128, 8, 512], BF16, tag="w2au_sb")
                    nc.sync.dma_start(out=w2au_sb,
                                      in_=W2au_d.rearrange("(a b) h -> b a h", b=128)[:, :, c2 * 512:(c2 + 1) * 512])
                    for tt in range(TTH):
                        gt = half * TTH + tt
                        xres = strm.tile([128, 512], F32, tag="xres")
                        nc.sync.dma_start(out=xres, in_=x_d[gt * 128:(gt + 1) * 128, c2 * 512:(c2 + 1) * 512])
                        so = strm.tile([128, 512], F32, tag="so")
                        l2 = psum.tile([128, 512], F32, tag="mm512")
                        for m in range(8):
                            nc.tensor.matmul(out=l2, lhsT=g1[m][:, tt * 128:(tt + 1) * 128],
                                             rhs=w2au_sb[:, m],
                                             start=(m == 0), stop=False)
                        nc.tensor.matmul(out=l2, lhsT=ones_row[:, 0:128],
                                         rhs=b2r_sb[:, c2 * 512:(c2 + 1) * 512],
                                         start=False, stop=True)
                        nc.vector.tensor_tensor(out=so, in0=l2, in1=xres, op=ALU.add)
                        nc.sync.dma_start(out=o_slots_d[gt * 128:(gt + 1) * 128, c2 * 512:(c2 + 1) * 512], in_=so)
                    # pooling accumulate (needs poolw -> only availableafter scores;
                    # we build poolw lazily right after the first half's scores
                    # are *not* complete... so pooling is deferred: see below)

            # scores assembly (after both halves)
            sc_in = load_small(sc_tok_d, [128, TT], "sc_in")
            sc_tok = small.tile([128, TT], F32, tag="sc_tok")
            nc.vector.tensor_tensor(out=sc_tok, in0=sc_sb, in1=sc_in, op=ALU.add)
            nc.vector.tensor_scalar_add(sc_tok, sc_tok, float(b2_scalar))
            scT_ps = psum.tile([128, 128], F32, tag="ps_small")
            nc.tensor.transpose(out=scT_ps[0:TT, 0:128], in_=sc_tok, identity=ident)
            scT = small.tile([TT, 128], F32, tag="scT")
            nc.vector.tensor_copy(out=scT, in_=scT_ps[0:TT, 0:128])
            nc.sync.dma_start(out=o_sc_d[:, :], in_=scT)

            # softmax over each 64-token group (2 per partition)
            scT3 = scT.rearrange("p (j k) -> p j k", j=2)
            mx = small.tile([TT, 2, 1], F32, tag="sm_mx")
            nc.vector.tensor_reduce(out=mx, in_=scT3, axis=AX.X, op=ALU.max)
            sme = small.tile([TT, 128], F32, tag="sme")
            nc.vector.tensor_tensor(out=sme.rearrange("p (j k) -> p j k", j=2), in0=scT3,
                                    in1=mx.broadcast_to([TT, 2, 64]), op=ALU.subtract)
            nc.scalar.activation(out=sme, in_=sme, func=AF.Exp)
            ssum = small.tile([TT, 2, 1], F32, tag="sm_sum")
            nc.vector.tensor_reduce(out=ssum, in_=sme.rearrange("p (j k) -> p j k", j=2),
                                    axis=AX.X, op=ALU.add)
            srcp = small.tile([TT, 2, 1], F32, tag="sm_rcp")
            nc.vector.reciprocal(out=srcp, in_=ssum)
            smw = small.tile([TT, 128], F32, tag="smw")
            nc.vector.tensor_tensor(out=smw.rearrange("p (j k) -> p j k", j=2),
                                    in0=sme.rearrange("p (j k) -> p j k", j=2),
                                    in1=srcp.broadcast_to([TT, 2, 64]), op=ALU.mult)
            smwT_ps = psum.tile([128, 128], F32, tag="ps_small")
            nc.tensor.transpose(out=smwT_ps[0:128, 0:TT], in_=smw, identity=ident[0:TT, 0:TT])
            smwT = small.tile([128, TT], F32, tag="smwT")
            nc.vector.tensor_copy(out=smwT, in_=smwT_ps[0:128, 0:TT])

            # pooling: pooled[b, :] = sum_t w[t, b] * slots_out[t, :]
            # slots_out is re-read from DRAM (it was just written above).
            pw_scr = pw_scr_a if side == "a" else pw_scr_r
            for tt in range(TT):
                pw = small.tile([128, B], F32, tag="poolw")
                nc.vector.memset(pw, 0.0)
                nc.vector.tensor_copy(out=pw[0:64, 2 * tt:2 * tt + 1], in_=smwT[0:64, tt:tt + 1])
                nc.vector.tensor_copy(out=pw[64:128, 2 * tt + 1:2 * tt + 2],
                                      in_=smwT[64:128, tt:tt + 1])
                nc.sync.dma_start(out=pw_scr[tt * 128:(tt + 1) * 128, :], in_=pw)
            for tt in range(TT):
                pwr = small.tile([128, B], R32, tag="poolw_r")
                nc.sync.dma_start(out=pwr, in_=pw_scr[tt * 128:(tt + 1) * 128, :].bitcast(R32))
                for c2 in range(2):
                    sres = strm.tile([128, 512], R32, tag="xres")
                    nc.sync.dma_start(out=sres,
                                      in_=o_slots_d[tt * 128:(tt + 1) * 128, c2 * 512:(c2 + 1) * 512].bitcast(R32))
                    pps = psum.tile([B, 512], F32, tag="pool_part")
                    nc.tensor.matmul(out=pps, lhsT=pwr[:], rhs=sres,
                                     start=True, stop=True)
                    nc.vector.tensor_tensor(out=pooled_sb[:, c2 * 512:(c2 + 1) * 512],
                                            in0=pooled_sb[:, c2 * 512:(c2 + 1) * 512],
                                            in1=pps, op=ALU.add)
            # transpose pooled to T layout right away: 8 tiles [128, B]
            pooledT = []
            for c in range(8):
                tp = psum.tile([128, 128], F32, tag="ps_small")
                nc.tensor.transpose(out=tp[0:128, 0:B], in_=pooled_sb[:, c * 128:(c + 1) * 128],
                                    identity=ident[0:B, 0:B])
                ts = small.tile([128, B], F32, tag=f"pT_{side}{c}")
                nc.vector.tensor_copy(out=ts, in_=tp[0:128, 0:B])
                pooledT.append(ts)
            return pooledT

        pooled_aT = do_side(xta_h, xta_l, xa, sca_tok, W1s_ad_h, W1s_ad_l, W1s_au,
                           W2ad_t, W2au, b2au_row, bias_ad, bias_au,
                           scalars["ad_b2"], o_a_slots, o_a_sc, "a")
        pooled_rT = do_side(xtr_h, xtr_l, xr, scr_tok, W1s_rd_h, W1s_rd_l, W1s_ru,
                           W2rd_t, W2ru, b2ru_row, bias_rd, bias_ru,
                           scalars["rd_b2"], o_r_slots, o_r_sc, "r")

        # ---------- stagnation ----------
        for m in range(8):
            nc.sync.dma_start(out=pooled_scr[m * 128:(m + 1) * 128, :], in_=pooled_aT[m][:])
            nc.sync.dma_start(out=pooled_scr[D + m * 128:D + (m + 1) * 128, :], in_=pooled_rT[m][:])
        paT_r, prT_r = [], []
        for m in range(8):
            t1 = small.tile([128, B], R32, tag=f"paTr{m}")
            nc.sync.dma_start(out=t1, in_=pooled_scr[m * 128:(m + 1) * 128, :].bitcast(R32))
            paT_r.append(t1)
            t2 = small.tile([128, B], R32, tag=f"prTr{m}")
            nc.sync.dma_start(out=t2, in_=pooled_scr[D + m * 128:D + (m + 1) * 128, :].bitcast(R32))
            prT_r.append(t2)
        st_in_r = cT_r + paT_r + prT_r
        st_h1 = bm_orient_mm(st_W1, D, st_in_r,
                             load_small(st_b1_t, [128, 8], "st_b1_sb"), AF.Gelu, "st_h1")
        st_W2_sb = load_small(st_W2_t, [128, 8], "st_W2_sb")
        st_acc = psum1.tile([128, B], F32, tag="acc_vec")
        for m in range(8):
            nc.tensor.matmul(out=st_acc[0:1, :], lhsT=st_W2_sb[:, m:m + 1], rhs=st_h1[m][:],
                             start=(m == 0), stop=(m == 7))
        st_sb = small.tile([1, B], F32, tag="st_sb")
        nc.scalar.activation(out=st_sb, in_=st_acc[0:1, :], func=AF.Identity,
                             bias=float(scalars["st_b2"]))
        nc.sync.dma_start(out=o_stag[:, :], in_=st_sb)

        # ---------- top-k (k=16 of 80) + gather ----------
        comb = small.tile([B, 80], F32, tag="comb")
        nc.sync.dma_start(out=comb[:, 0:A], in_=arch_sc[:, :])
        nc.sync.dma_start(out=comb[:, A:80], in_=o_a_sc.rearrange("i (j k) -> (i j) k", j=2))
        iota_i = small.tile([B, 80], mybir.dt.int32, tag="iota_i")
        nc.gpsimd.iota(iota_i, pattern=[[1, 80]], base=0, channel_multiplier=0)
        iota80 = small.tile([B, 80], F32, tag="iota80")
        nc.vector.tensor_copy(out=iota80, in_=iota_i)
        neginf = small.tile([B, 80], F32, tag="neginf")
        nc.vector.memset(neginf, -1e30)
        big = small.tile([B, 80], F32, tag="big")
        nc.vector.memset(big, 127.0)
        tvals = small.tile([B, A], F32, tag="tvals")
        tidx = small.tile([B, A], F32, tag="tidx")
        eqm = small.tile([B, 80], mybir.dt.int32, tag="eqm")
        cand = small.tile([B, 80], F32, tag="cand")
        for j in range(A):
            nc.vector.tensor_reduce(out=tvals[:, j:j + 1], in_=comb, axis=AX.X, op=ALU.max)
            nc.vector.tensor_tensor(out=eqm, in0=comb,
                                    in1=tvals[:, j:j + 1].broadcast_to([B, 80]), op=ALU.is_ge)
            nc.vector.select(out=cand, mask=eqm, on_true=iota80, on_false=big)
            nc.vector.tensor_reduce(out=tidx[:, j:j + 1], in_=cand, axis=AX.X, op=ALU.min)
            nc.vector.tensor_tensor(out=eqm, in0=iota80,
                                    in1=tidx[:, j:j + 1].broadcast_to([B, 80]), op=ALU.is_equal)
            nc.vector.select(out=comb, mask=eqm, on_true=neginf, on_false=comb)
        nc.sync.dma_start(out=o_top_vals[:, :], in_=tvals)

        nc.sync.dma_start(out=idx_scratch[:, :], in_=tidx)
        idx_bc = small.tile([80, B * A], F32, tag="idx_bc")
        bc_src = bass.AP(tensor=idx_scratch, offset=0, ap=[[0, 80], [1, B * A]])
        nc.sync.dma_start(out=idx_bc, in_=bc_src)
        iota_a_i = small.tile([A, B * A], mybir.dt.int32, tag="iota_a_i")
        nc.gpsimd.iota(iota_a_i, pattern=[[0, B * A]], base=0, channel_multiplier=1)
        s_arch_f = small.tile([A, B * A], F32, tag="s_arch_f")
        nc.vector.tensor_copy(out=s_arch_f, in_=iota_a_i)
        nc.vector.tensor_tensor(out=s_arch_f, in0=s_arch_f, in1=idx_bc[0:A, :], op=ALU.is_equal)
        iota_s_i = small.tile([KSLOT, B * A], mybir.dt.int32, tag="iota_s_i")
        nc.gpsimd.iota(iota_s_i, pattern=[[0, B * A]], base=A, channel_multiplier=1)
        s_slot_f = small.tile([KSLOT, B * A], F32, tag="s_slot_f")
        nc.vector.tensor_copy(out=s_slot_f, in_=iota_s_i)
        nc.vector.tensor_tensor(out=s_slot_f, in0=s_slot_f, in1=idx_bc[0:KSLOT, :], op=ALU.is_equal)
        nc.sync.dma_start(out=s_scr_a[:, :], in_=s_arch_f)
        nc.sync.dma_start(out=s_scr_s[:, :], in_=s_slot_f)
        s_arch = small.tile([A, B * A], R32, tag="s_arch")
        nc.sync.dma_start(out=s_arch, in_=s_scr_a[:, :].bitcast(R32))
        s_slot = small.tile([KSLOT, B * A], R32, tag="s_slot")
        nc.sync.dma_start(out=s_slot, in_=s_scr_s[:, :].bitcast(R32))
        for b in range(B):
            arch_b = strm1.tile([A, D], R32, tag="arch_b")
            nc.sync.dma_start(out=arch_b, in_=arch_slots[b].bitcast(R32))
            slots_b = strm1.tile([KSLOT, D], R32, tag="slots_b")
            nc.sync.dma_start(out=slots_b, in_=o_a_slots[b * KSLOT:(b + 1) * KSLOT, :].bitcast(R32))
            gsb = strm1.tile([A, D], F32, tag="gsb")
            for c2 in range(2):
                gps = psum1.tile([A, 512], F32, tag="gps")
                nc.tensor.matmul(out=gps, lhsT=s_arch[:, b * A:(b + 1) * A],
                                 rhs=arch_b[:, c2 * 512:(c2 + 1) * 512],
                                 start=True, stop=False)
                nc.tensor.matmul(out=gps, lhsT=s_slot[:, b * A:(b + 1) * A],
                                 rhs=slots_b[:, c2 * 512:(c2 + 1) * 512],
                                 start=False, stop=True)
                nc.vector.tensor_copy(out=gsb[:, c2 * 512:(c2 + 1) * 512], in_=gps)
            nc.sync.dma_start(out=o_top_slots[b], in_=gsb)

    return nc


def kernel(**inputs):
    from concourse.bass_utils import run_bass_kernel_spmd
    import ml_dtypes

    bf16 = ml_dtypes.bfloat16
    f32 = lambda k: np.asarray(inputs[k], dtype=np.float32)
    FB = NCORES * B

    scalars = {
        "ad_b2": float(np.asarray(inputs["ad_b2"]).reshape(-1)[0]),
        "rd_b2": float(np.asarray(inputs["rd_b2"]).reshape(-1)[0]),
        "s_b2": float(np.asarray(inputs["s_b2"]).reshape(-1)[0]),
        "st_b2": float(np.asarray(inputs["st_b2"]).reshape(-1)[0]),
    }

    def col_t(v, nm):
        return np.ascontiguousarray(v.reshape(nm, 128).T.astype(np.float32))

    au_W1, ad_W1 = f32("au_W1"), f32("ad_W1")
    ru_W1, rd_W1 = f32("ru_W1"), f32("rd_W1")
    W1s_ad_h, W1s_ad_l = _split(np.ascontiguousarray(ad_W1[0:D]))
    W1s_rd_h, W1s_rd_l = _split(np.ascontiguousarray(rd_W1[0:D]))
    shared = {
        "Wp": f32("Wp"), "Wo": f32("Wo"),
        "bp_t": col_t(f32("bp"), 8), "bo_t": col_t(f32("bo"), 8),
        "Wctx_au": np.ascontiguousarray(au_W1[D:4 * D]),
        "Wctx_ad": np.ascontiguousarray(ad_W1[D:4 * D]),
        "Wctx_ru": np.ascontiguousarray(ru_W1[D:4 * D]),
        "Wctx_rd": np.ascontiguousarray(rd_W1[D:4 * D]),
        "b1au_t": col_t(f32("au_b1"), 8), "b1ad_t": col_t(f32("ad_b1"), 16),
        "b1ru_t": col_t(f32("ru_b1"), 8), "b1rd_t": col_t(f32("rd_b1"), 16),
        "W1s_au": np.ascontiguousarray(au_W1[0:D]),
        "W1s_ru": np.ascontiguousarray(ru_W1[0:D]),
        "W1s_ad_h": W1s_ad_h, "W1s_ad_l": W1s_ad_l,
        "W1s_rd_h": W1s_rd_h, "W1s_rd_l": W1s_rd_l,
        "W2au": f32("au_W2").astype(bf16), "W2ru": f32("ru_W2").astype(bf16),
        "b2au_row": f32("au_b2").reshape(1, D).astype(bf16),
        "b2ru_row": f32("ru_b2").reshape(1, D).astype(bf16),
        "W2ad_t": col_t(f32("ad_W2").reshape(-1), 16),
        "W2rd_t": col_t(f32("rd_W2").reshape(-1), 16),
        "s_W1": f32("s_W1"), "st_W1": f32("st_W1"),
        "s_b1_t": col_t(f32("s_b1"), 8), "st_b1_t": col_t(f32("st_b1"), 8),
        "s_W2_t": col_t(f32("s_W2").reshape(-1), 8),
        "st_W2_t": col_t(f32("st_W2").reshape(-1), 8),
    }

    a_slots = f32("approach_slots"); r_slots = f32("rule_slots")
    a_sc = f32("approach_scores"); r_sc = f32("rule_scores")
    arch_slots_full = f32("archive_slots"); arch_sc_full = f32("archive_scores")
    ctx_full = f32("context_summary"); cp_full = f32("chosen_probe_emb")
    ob_full = f32("observation_emb")

    in_maps = []
    for ci in range(NCORES):
        sl = slice(ci * B, (ci + 1) * B)
        xa_ = np.ascontiguousarray(a_slots[sl].reshape(T, D))
        xr_ = np.ascontiguousarray(r_slots[sl].reshape(T, D))
        xta_h_, xta_l_ = _split(np.ascontiguousarray(xa_.T))
        xtr_h_, xtr_l_ = _split(np.ascontiguousarray(xr_.T))
        m = {
            "xa": xa_, "xr": xr_,
            "xta_h": xta_h_, "xta_l": xta_l_, "xtr_h": xtr_h_, "xtr_l": xtr_l_,
            "sca_tok": np.ascontiguousarray(a_sc[sl].reshape(TT, 128).T),
            "scr_tok": np.ascontiguousarray(r_sc[sl].reshape(TT, 128).T),
            "arch_slots": np.ascontiguousarray(arch_slots_full[sl]),
            "arch_sc": np.ascontiguousarray(arch_sc_full[sl]),
            "ctxT": np.ascontiguousarray(ctx_full[sl].T),
            "cpT": np.ascontiguousarray(cp_full[sl].T),
            "obT": np.ascontiguousarray(ob_full[sl].T),
        }
        m.update(shared)
        in_maps.append(m)

    nc = build_nc(scalars)
    nc.compile()
    import os
    trace = os.environ.get("BELIEF_TRACE") == "1"
    kwargs = {}
    if trace:
        kwargs = dict(trace=True, tmpdir="/tmp/belief_ntff")
    res = run_bass_kernel_spmd(nc, in_maps, list(range(NCORES)), **kwargs)
    global LAST_EXEC_NS
    LAST_EXEC_NS = res.exec_time_ns
    if os.environ.get("BELIEF_BENCH") == "1":
        import time as _time
        for _i in range(3):
            _t0 = _time.time()
            res = run_bass_kernel_spmd(nc, in_maps, list(range(NCORES)), **kwargs)
            print(f"warm run {_i}: {_time.time() - _t0:.3f}s", flush=True)

    new_a_slots = np.empty((FB, KSLOT, D), np.float32)
    new_r_slots = np.empty((FB, KSLOT, D), np.float32)
    new_a_sc = np.empty((FB, KSLOT), np.float32)
    new_r_sc = np.empty((FB, KSLOT), np.float32)
    top_slots = np.empty((FB, A, D), np.float32)
    top_vals = np.empty((FB, A), np.float32)
    surprise = np.empty((FB, 1), np.float32)
    stag = np.empty((FB, 1), np.float32)
    for ci in range(NCORES):
        r = res.results[ci]
        sl = slice(ci * B, (ci + 1) * B)
        new_a_slots[sl] = r["o_a_slots"].reshape(B, KSLOT, D)
        new_r_slots[sl] = r["o_r_slots"].reshape(B, KSLOT, D)
        new_a_sc[sl] = r["o_a_sc"].reshape(T).reshape(B, KSLOT)
        new_r_sc[sl] = r["o_r_sc"].reshape(T).reshape(B, KSLOT)
        top_slots[sl] = r["o_top_slots"]
        top_vals[sl] = r["o_top_vals"]
        surprise[sl] = r["o_surprise"].reshape(B, 1)
        stag[sl] = r["o_stag"].reshape(B, 1)
    return (new_a_slots, new_a_sc, new_r_slots, new_r_sc,
            top_slots, top_vals, surprise, stag)
